# revision 65
# baseline (speedup 1.0000x reference)
"""Sparse attention (ProbSparse-style) Trainium2 Bass kernel.

Problem (per batch element b, data-parallel over 8 NeuronCores):
  Q = x @ Wq.T ; K = x @ Wk.T ; V = x @ Wv.T            [L=2048, D=512]
  QK_sample[l,s] = Q[l] . K[index_sample[l,s]]           [L, 40]
  M[l] = max_s QK_sample - sum_s QK_sample / L
  sel = top40(M)  (as a set; the reference scatter makes order irrelevant)
  scores = Q[sel] @ K.T / sqrt(D); attn = softmax(scores)
  ctx = broadcast(mean(V)); ctx[sel] = attn @ V

Numerics strategy (top-40 boundary gaps are as small as 0.02 in M):
  - Q/K/V projections are single-term bf16 (approx only). No exact K is
    ever built: the exact stage evaluates scores = x_cand @ A @ x^T with
    A = Wq^T Wk precomputed on the host in f64, where x_cand is the
    gathered f32 input and the A-product is fp32 on PE, then multiplied
    against the bf16x2 (hi/lo) split of x^T with a 3-term product
    (~4e-4 absolute error, well inside the 0.02 gap budget).
  - Approx M for ALL rows: per (lc, jb) block S = Qb Kb^T accumulates in
    PSUM, the scalar engine drains it to bf16 SBUF, the vector engine
    applies the bf16 0/1 sample mask in 2x packed mode and max8-reduces
    (multiply-mask max is safe: sampled max > 0 w.p. 1-2^-40; dup-count
    correction is deferred to the exact stage). The -sum_s/L term is
    omitted from approx M and absorbed into DELTA (exact stage uses the
    full formula via gathered mask+count rows).
  - approx-T40 is computed exactly with vector max8/match_replace (no
    GPSIMD kth_largest: its attn-library ucode reload cost ~50us of
    dead serial time): per-chunk top-16 of M^T [16,128], one-hot-matmul
    unwrap + PE transposes to a [1,256] row, 5 rounds of
    max8/match_replace. Union of per-chunk top-16 contains the global
    top-40 w.p. 1-2e-8; a miss only widens the candidate band.
  - Candidates = { M_approx >= T40 - DELTA }, DELTA=2.2 covers ~8 sigma
    of bf16 dot error (x2 sides) plus the omitted sum term; measured
    rank-40 to rank-64 M gap is 2.5-4.8 so the candidate count stays
    well under the 128-slot budget (GPSIMD sparse_gather compaction).
  - Exact stage on <= 128 candidate rows: indirect-gather x/mask/count
    rows, Y = x_cand @ A (fp32), S_cand = Y @ x^T (bf16x2 3-term),
    masked TTR -> exact M_cand -> exact top-40 threshold (vector
    max8 rounds) -> softmax (no max-subtraction; scores*scale is O(10))
    -> upd = attn @ V -> indirect scatter of the 40 selected rows into
    ctx (bounds_check skips the rest). ctx is pre-filled with the
    host-computed vmean broadcast.
Scheduling: jb-major 1-term projections start as soon as the first
1024-column x slices land; the V projection and vmean broadcast are
interleaved through the threshold/compaction serial tail to keep PE
busy (and HAM warm); expT transposes pipeline with the upd
accumulation while the vector engine runs the candidate top-40.

kernel(**inputs) accepts the FULL inputs and returns the FULL
[8, 2048, 512] f32 output; batch is sharded over 8 cores.
"""

import math

import numpy as np
import ml_dtypes

import concourse.bacc as bacc
import concourse.bass as bass
import concourse.mybir as mybir
import concourse.tile as tile
from concourse.bass_utils import run_bass_kernel_spmd

P = 128
L = 2048
D = 512
B = 8
NL = L // P        # 16 query chunks
ND = D // P        # 4 feature chunks
NJ = L // 512      # 4 key blocks of 512
NT = 40
SCALE = 1.0 / math.sqrt(D)
# candidate band below approx T40: covers 2x bf16 dot error (~8 sigma =
# 1.5) plus the omitted -sum_s/L term in approx M (|sum/L| <= ~0.25
# per row at 3.5 sigma, both directions -> +0.5)
DELTA = 2.2
NEG = -3.0e38
SKIP_IDX = 99999.0  # scatter index sentinel (> bounds_check -> row skipped)

f32 = mybir.dt.float32
bf16 = mybir.dt.bfloat16
u8 = mybir.dt.uint8
i32 = mybir.dt.int32
u32 = mybir.dt.uint32
AX = mybir.AxisListType
OP = mybir.AluOpType
ACTF = mybir.ActivationFunctionType


def build():
    nc = bacc.Bacc("TRN2", target_bir_lowering=False)

    x_d = nc.dram_tensor("x_nat", [L, D], f32, kind="ExternalInput")
    xth_d = nc.dram_tensor("xTh", [D, L], bf16, kind="ExternalInput")
    xtl_d = nc.dram_tensor("xTl", [D, L], bf16, kind="ExternalInput")
    # bf16 weight tiles packed into one wide row-major tensor:
    # [wqh|wkh|wvh], each 4 tiles of 512 cols
    wcat_d = nc.dram_tensor("wcat", [P, 12 * 512], bf16, kind="ExternalInput")
    vmn_d = nc.dram_tensor("vmeanr", [1, D], f32, kind="ExternalInput")
    # A = Wq^T @ Wk (host f64): scores = x_cand @ A @ x^T, so no exact K
    # projection is ever needed on-device
    acat_d = nc.dram_tensor("Acat", [P, 4 * D], f32, kind="ExternalInput")
    maskb_d = nc.dram_tensor("maskb", [L, L], bf16, kind="ExternalInput")
    ident_d = nc.dram_tensor("identf", [P, P], f32, kind="ExternalInput")
    qidx_d = nc.dram_tensor("qidxf", [P, 16], f32, kind="ExternalInput")
    perm_d = nc.dram_tensor("perm16", [16, 8 * P], f32, kind="ExternalInput")
    mask_d = nc.dram_tensor("mask01", [L, L], u8, kind="ExternalInput")
    cnt_d = nc.dram_tensor("countf", [L, L], u8, kind="ExternalInput")
    ctx_d = nc.dram_tensor("ctx", [L, D], f32, kind="ExternalOutput")

    with tile.TileContext(nc) as tc:
        with (
            tc.tile_pool(name="const", bufs=1) as cst,
            tc.tile_pool(name="proj", bufs=1) as proj,       # KT/KTb/QTb/V resident
            tc.tile_pool(name="mstuff", bufs=1) as mst,      # M / topk / sel smalls
            tc.tile_pool(name="mstream", bufs=3) as mstr,    # mask chunks
            tc.tile_pool(name="scr", bufs=3) as scr,         # TTR scratch
            tc.tile_pool(name="acc", bufs=2) as accp,        # per-chunk accums
            tc.tile_pool(name="cand", bufs=1) as cnd,        # exact-stage tiles
            tc.tile_pool(name="ps", bufs=3, space="PSUM") as ps,
            tc.tile_pool(name="ps_s", bufs=4, space="PSUM") as ps_s,  # S_cand (held)
            tc.tile_pool(name="dram", bufs=1, space="DRAM") as drp,
        ):
            # ---------------- constants ----------------
            # sparse_gather is the only library-tracked GPSIMD op left;
            # preload its (small) library before the serial tail
            from concourse import library_config
            nc.gpsimd.load_library(library_config.sparse_gather)
            ident = cst.tile([P, P], f32, tag="ident")
            nc.sync.dma_start(ident[:], ident_d[:])
            ones_r1 = cst.tile([1, P], f32, tag="ones_r1")
            nc.vector.memset(ones_r1[:], 1.0)
            negone = cst.tile([P, 1], f32, tag="negone")
            nc.vector.memset(negone[:], -1.0)
            negbig = cst.tile([P, 1], f32, tag="negbig")
            nc.vector.memset(negbig[:], NEG)
            big9 = cst.tile([P, 1], f32, tag="big9")
            nc.vector.memset(big9[:], SKIP_IDX)
            perm16 = cst.tile([16, 8 * P], f32, tag="perm16")
            nc.sync.dma_start(perm16[:], perm_d[:])
            qidx_f = cst.tile([P, 16], f32, tag="qidx_f")    # value p + 128*c
            nc.sync.dma_start(qidx_f[:], qidx_d[:])
            # vmean = mean(x) @ Wv.T is shipped from the host; broadcast it
            # to 128 rows and start the 4MB ctx init writes immediately so
            # they ride otherwise-idle DMA bandwidth for the whole kernel
            vmn = cst.tile([1, D], f32, tag="vmn")
            nc.sync.dma_start(vmn[:], vmn_d[:])

            # resident projection outputs
            KTb = [proj.tile([P, L], bf16, tag=f"KTb{ic}", name=f"KTb{ic}") for ic in range(ND)]
            QTb = [proj.tile([P, L], bf16, tag=f"QTb{ic}", name=f"QTb{ic}") for ic in range(ND)]
            Vb = [proj.tile([P, D], bf16, tag=f"Vb{jc}", name=f"Vb{jc}") for jc in range(NL)]

            with tc.tile_pool(name="xw", bufs=1) as xw:
                # ---------------- phase 0: loads ----------------
                xTh = [xw.tile([P, L], bf16, tag=f"xTh{dc}", name=f"xTh{dc}") for dc in range(ND)]
                xTl = [xw.tile([P, L], bf16, tag=f"xTl{dc}", name=f"xTl{dc}") for dc in range(ND)]
                wcat = xw.tile([P, 12 * 512], bf16, tag="wcat")
                acat = xw.tile([P, 4 * D], f32, tag="acat")
                # weight-tile views into the packed wcat: [wqh|wkh|wvh]
                def wview(group, dc):
                    off = group * 4 * 512 + dc * 512
                    return wcat[:, off : off + 512]
                wqh = [wview(0, dc) for dc in range(ND)]
                wkh = [wview(1, dc) for dc in range(ND)]
                wvh = [wview(2, dc) for dc in range(ND)]
                Arc = [acat[:, dc * 512 : (dc + 1) * 512] for dc in range(ND)]
                # DMA order: wqh + xTh first (Q projection starts earliest),
                # then wkh, x-lo, wvh, then the f32 Acat (tail only).
                # 1024-col chunks = 2KB rows, spread across queues.
                nc.sync.dma_start(wcat[:, 0:1024], wcat_d[:, 0:1024])
                nc.sync.dma_start(wcat[:, 1024:2048], wcat_d[:, 1024:2048])
                for dc in range(ND):
                    sl = slice(dc * P, (dc + 1) * P)
                    nc.sync.dma_start(xTh[dc][:, 0:1024], xth_d[sl, 0:1024])
                    nc.sync.dma_start(xTh[dc][:, 1024:2048], xth_d[sl, 1024:2048])
                for c0 in range(2048, 4096, 1024):       # wkh
                    nc.sync.dma_start(wcat[:, c0 : c0 + 1024], wcat_d[:, c0 : c0 + 1024])
                for dc in range(ND):
                    sl = slice(dc * P, (dc + 1) * P)
                    nc.sync.dma_start(xTl[dc][:, 0:1024], xtl_d[sl, 0:1024])
                    nc.sync.dma_start(xTl[dc][:, 1024:2048], xtl_d[sl, 1024:2048])
                for c0 in range(4096, 6144, 1024):       # wvh
                    nc.sync.dma_start(wcat[:, c0 : c0 + 1024], wcat_d[:, c0 : c0 + 1024])
                nc.sync.dma_start(acat[:, 0:1024], acat_d[:, 0:1024])
                nc.sync.dma_start(acat[:, 1024:2048], acat_d[:, 1024:2048])

                # ---------------- phase 1: projections ----------------
                # Q first (single bf16 term, needs only wqh+xTh), jb-major;
                # then K 3-term jb-major so S blocks can start after K jb=0.
                for jb in range(NJ):
                    jsl = slice(jb * 512, (jb + 1) * 512)
                    for ic in range(ND):
                        isl = slice(ic * P, (ic + 1) * P)
                        pq = ps.tile([P, 512], f32, tag="blk")
                        for dc in range(ND):
                            nc.tensor.matmul(
                                pq[:], wqh[dc][:, isl], xTh[dc][:, jsl],
                                start=(dc == 0), stop=(dc == ND - 1),
                            )
                        nc.scalar.copy(QTb[ic][:, jsl], pq[:])
                # K approx: single bf16 term (the exact stage goes through
                # A = Wq^T Wk and never needs an exact K)
                for jb in range(NJ):
                    jsl = slice(jb * 512, (jb + 1) * 512)
                    for ic in range(ND):
                        isl = slice(ic * P, (ic + 1) * P)
                        pk = ps.tile([P, 512], f32, tag="blk")
                        for dc in range(ND):
                            nc.tensor.matmul(
                                pk[:], wkh[dc][:, isl], xTh[dc][:, jsl],
                                start=(dc == 0), stop=(dc == ND - 1),
                            )
                        nc.scalar.copy(KTb[ic][:, jsl], pk[:])

                # ---------------- phase 2: approx M (bf16 S) ----------------
                # per (lc, jb) block: ONE fused TTR (masked product -> bf16
                # scratch, fused max accum). The -sum_s/L term is omitted in
                # the approx M (absorbed into DELTA); the exact stage still
                # uses the full formula.
                M_all = mst.tile([P, 16], f32, tag="M_all")
                amax_all = mst.tile([P, NL * NJ * 8], f32, tag="amax_all")
                for lc in range(NL):
                    lsl = slice(lc * P, (lc + 1) * P)
                    mk = mstr.tile([P, L], bf16, tag="mk")
                    nc.sync.dma_start(mk[:], maskb_d[lsl, :])
                    for jb in range(NJ):
                        jsl = slice(jb * 512, (jb + 1) * 512)
                        k = lc * NJ + jb
                        pss = ps_s.tile([P, 512], f32, tag="psSc", name="pssa")
                        for ic in range(ND):
                            nc.tensor.matmul(
                                pss[:], QTb[ic][:, lsl], KTb[ic][:, jsl],
                                start=(ic == 0), stop=(ic == ND - 1),
                            )
                        # scalar engine drains PSUM to bf16 SBUF so the DVE
                        # mask-multiply runs in 2x packed mode (both operands
                        # bf16 SBUF); reduce_max is 1x regardless.
                        s0 = scr.tile([P, 512], bf16, tag="s0t")
                        nc.scalar.copy(s0[:], pss[:])
                        s1 = scr.tile([P, 512], bf16, tag="scrt")
                        nc.vector.tensor_tensor(
                            out=s1[:], in0=s0[:], in1=mk[:, jsl], op=OP.mult
                        )
                        nc.vector.max(
                            out=amax_all[:, k * 8 : (k + 1) * 8], in_=s1[:]
                        )

                # ---------------- phase 3: approx top-40 -> candidates ------
                # V projection (single bf16 term: the upd matmul consumes
                # bf16 anyway) is interleaved through the threshold/compaction
                # chain in chunks so PE stays busy (and HAM stays warm) while
                # the vector engine and GPSIMD work through the serial tail.
                def v_chunks(lo, hi):
                    for lc in range(lo, hi):
                        lsl = slice(lc * P, (lc + 1) * P)
                        pv = ps.tile([P, 512], f32, tag="blk")
                        for dc in range(ND):
                            nc.tensor.matmul(
                                pv[:], xTh[dc][:, lsl], wvh[dc][:],
                                start=(dc == 0), stop=(dc == ND - 1),
                            )
                        nc.scalar.copy(Vb[lc][:], pv[:])

                # V 0..7 first: PE covers the vector drain of the last S
                # blocks + the M_all combine without going HAM-cold
                v_chunks(0, 8)

                nc.vector.reduce_max(
                    M_all[:],
                    amax_all[:].rearrange("p (c j) -> p c j", j=NJ * 8),
                    axis=AX.X,
                )

                # exact T40 of M_approx without GPSIMD kth_largest (its attn
                # library reload cost ~50us of dead time on the serial tail):
                # per-chunk top-16 via vector max8/match_replace on M^T
                # [16,128], union (256 vals) holds the global top-40 w.p.
                # 1-2e-8, pack union into one [1,256] row via one-hot matmul
                # unwrap + transposes, then 5 rounds max8/match_replace.
                pmt = ps.tile([16, P], f32, tag="blk", name="pmt")
                nc.tensor.transpose(pmt[:16, :P], M_all[:], ident[:])
                MT = mst.tile([16, P], f32, tag="MT")
                nc.vector.tensor_copy(MT[:], pmt[:16, :P])
                w16 = mst.tile([16, 16], f32, tag="w16")
                nc.vector.max(out=w16[:, 0:8], in_=MT[:])
                nc.vector.match_replace(
                    out=MT[:], in_to_replace=w16[:, 0:8],
                    in_values=MT[:], imm_value=NEG,
                )
                nc.vector.max(out=w16[:, 8:16], in_=MT[:])

                v_chunks(8, 10)

                # unwrap w16 [16,16] -> two [128,1] columns (one-hot matmuls),
                # then -> [1,256] row via two PE transposes
                pcu = ps.tile([P, 2], f32, tag="blk", name="pcu")
                for f in range(8):
                    nc.tensor.matmul(
                        pcu[:P, 0:1], perm16[:, f * P : (f + 1) * P],
                        w16[:, f : f + 1],
                        start=(f == 0), stop=(f == 7),
                    )
                for f in range(8):
                    nc.tensor.matmul(
                        pcu[:P, 1:2], perm16[:, f * P : (f + 1) * P],
                        w16[:, 8 + f : 9 + f],
                        start=(f == 0), stop=(f == 7),
                    )
                crow = mst.tile([P, 2], f32, tag="crow")
                nc.vector.tensor_copy(crow[:], pcu[:P, :2])
                pr1 = ps.tile([1, P], f32, tag="blk", name="pr1")
                nc.tensor.transpose(pr1[:1, :P], crow[:, 0:1], ident[:])
                wrow = mst.tile([1, 2 * P], f32, tag="wrow")
                nc.vector.tensor_copy(wrow[:, 0:P], pr1[:1, :P])
                pr2 = ps.tile([1, P], f32, tag="blk", name="pr2")
                nc.tensor.transpose(pr2[:1, :P], crow[:, 1:2], ident[:])
                nc.vector.tensor_copy(wrow[:, P : 2 * P], pr2[:1, :P])

                v_chunks(10, 12)

                etop40 = mst.tile([1, NT], f32, tag="etop40")
                for r in range(5):
                    nc.vector.max(out=etop40[:, 8 * r : 8 * r + 8], in_=wrow[:])
                    if r < 4:
                        nc.vector.match_replace(
                            out=wrow[:], in_to_replace=etop40[:, 8 * r : 8 * r + 8],
                            in_values=wrow[:], imm_value=NEG,
                        )
                ptb = ps.tile([P, 1], f32, tag="blk")
                nc.tensor.matmul(
                    ptb[:P, :1], ones_r1[:], etop40[:, NT - 1 : NT],
                    start=True, stop=True,
                )
                tbc = mst.tile([P, 1], f32, tag="tbc")
                nc.vector.tensor_copy(tbc[:], ptb[:P, :1])

                # selmask = (M - T40) >= -DELTA, one fused op
                selmask = mst.tile([P, 16], u8, tag="selmask")
                nc.vector.tensor_scalar(
                    selmask[:], M_all[:], tbc[:], -DELTA,
                    op0=OP.subtract, op1=OP.is_ge,
                )
                midx = mst.tile([P, 16], f32, tag="midx")
                nc.vector.tensor_copy(midx[:], negone[:].to_broadcast([P, 16]))
                nc.vector.copy_predicated(midx[:], selmask[:], qidx_f[:])

                pwr = ps.tile([16, P], f32, tag="blk", name="pwr")
                nc.tensor.transpose(pwr[:16, :P], midx[:], ident[:])
                wrap_in = mst.tile([16, P], f32, tag="wrap_in")
                nc.vector.tensor_copy(wrap_in[:], pwr[:16, :P])
                spg = mst.tile([16, 8], f32, tag="spg")
                nfound = mst.tile([1, 1], u32, tag="nfound")
                nc.gpsimd.sparse_gather(out=spg[:], in_=wrap_in[:], num_found=nfound[:])

                v_chunks(12, 14)

                spg_cl = mst.tile([16, 8], f32, tag="spg_cl")
                nc.vector.tensor_scalar_max(spg_cl[:], spg[:], 0.0)
                nc.vector.tensor_scalar_min(spg_cl[:], spg_cl[:], float(L - 1))

                # unwrap [16,8] -> [128,1] with 8 tiny one-hot matmuls
                # (perm16[p, f*128+u] = 1 iff u == p + 16*f, shipped constant)
                pcq = ps.tile([P, 1], f32, tag="blk", name="pcq")
                for f in range(8):
                    nc.tensor.matmul(
                        pcq[:P, :1], perm16[:, f * P : (f + 1) * P],
                        spg_cl[:, f : f + 1],
                        start=(f == 0), stop=(f == 7),
                    )
                candq_f = mst.tile([P, 1], f32, tag="candq_f")
                nc.vector.tensor_copy(candq_f[:], pcq[:P, :1])
                candq_i = mst.tile([P, 1], i32, tag="candq_i")
                nc.vector.tensor_copy(candq_i[:], pcq[:P, :1])

                nf_f = mst.tile([1, 1], f32, tag="nf_f")
                nc.vector.tensor_copy(nf_f[:], nfound[:])
                pnb = ps.tile([P, 1], f32, tag="blk")
                nc.tensor.matmul(pnb[:P, :1], ones_r1[:], nf_f[:], start=True, stop=True)
                nbc = mst.tile([P, 1], f32, tag="nbc")
                nc.vector.tensor_copy(nbc[:], pnb[:P, :1])
                invalid = mst.tile([P, 1], u8, tag="invalid")
                nc.vector.tensor_tensor(
                    out=invalid[:], in0=qidx_f[:, 0:1], in1=nbc[:], op=OP.is_ge
                )

                v_chunks(14, 16)

                # ctx init writes: masks are done streaming by now, DMA is
                # otherwise idle, and these finish before the final scatter
                pvb = ps.tile([P, D], f32, tag="blk", name="pvb")
                nc.tensor.matmul(pvb[:], ones_r1[:], vmn[:], start=True, stop=True)
                vmean_bc = cst.tile([P, D], f32, tag="vmean_bc")
                nc.vector.tensor_copy(vmean_bc[:], pvb[:])
                for jc in range(NL):
                    nc.sync.dma_start(ctx_d[jc * P : (jc + 1) * P, :], vmean_bc[:])

                # ---------------- phase 4a: exact candidates ----------------
                x_cand = cnd.tile([P, D], f32, tag="x_cand")
                nc.gpsimd.indirect_dma_start(
                    out=x_cand[:], out_offset=None, in_=x_d[:],
                    in_offset=bass.IndirectOffsetOnAxis(ap=candq_i[:, :1], axis=0),
                )
                xcT = [cnd.tile([P, P], f32, tag=f"xcT{dc}", name=f"xcT{dc}") for dc in range(ND)]
                for dc in range(ND):
                    pxc = ps.tile([P, P], f32, tag="blk")
                    nc.tensor.transpose(
                        pxc[:P, :P], x_cand[:, dc * P : (dc + 1) * P], ident[:]
                    )
                    nc.vector.tensor_copy(xcT[dc][:], pxc[:P, :P])

                # Y^T = (x_cand @ A)^T in f32 via PE, then bf16 hi/lo split
                # for the 3-term S_cand product against xTh/xTl
                YTh = [cnd.tile([P, P], bf16, tag=f"YTh{ic}", name=f"YTh{ic}") for ic in range(ND)]
                YTl = [cnd.tile([P, P], bf16, tag=f"YTl{ic}", name=f"YTl{ic}") for ic in range(ND)]
                for ic in range(ND):
                    isl = slice(ic * P, (ic + 1) * P)
                    pqc = ps.tile([P, P], f32, tag="blk")
                    for dc in range(ND):
                        nc.tensor.matmul(
                            pqc[:P, :P], Arc[dc][:, isl], xcT[dc][:],
                            start=(dc == 0), stop=(dc == ND - 1),
                        )
                    nc.vector.tensor_copy(YTh[ic][:], pqc[:P, :P])
                    nc.vector.tensor_tensor(
                        out=YTl[ic][:], in0=pqc[:P, :P], in1=YTh[ic][:],
                        op=OP.subtract,
                    )

                gm = cnd.tile([P, L], u8, tag="gm")
                nc.gpsimd.indirect_dma_start(
                    out=gm[:], out_offset=None, in_=mask_d[:],
                    in_offset=bass.IndirectOffsetOnAxis(ap=candq_i[:, :1], axis=0),
                )
                gc = cnd.tile([P, L], u8, tag="gc")
                nc.gpsimd.indirect_dma_start(
                    out=gc[:], out_offset=None, in_=cnt_d[:],
                    in_offset=bass.IndirectOffsetOnAxis(ap=candq_i[:, :1], axis=0),
                )

                # ------- phase 4a/4b fused: S_cand + softmax + update -------
                # each jb iteration: 12 S_cand matmuls, masked TTRs, EXP(jb),
                # then the expT transposes + upd matmuls for jb-1's exp
                # columns. The S_cand matmuls give the scalar PSUM drains
                # plenty of slack, so the expT/upd pipeline has no bubbles.
                psS = []
                cmax = cnd.tile([P, NJ], f32, tag="cmax")
                csum = cnd.tile([P, NJ], f32, tag="csum")
                exp_sb = cnd.tile([P, L], f32, tag="exp_sb")
                sume4 = cnd.tile([P, NJ], f32, tag="sume4")
                expT = [cnd.tile([P, P], bf16, tag=f"expT{jc}", name=f"expT{jc}") for jc in range(NL)]
                pu = ps.tile([P, 512], f32, tag="blk", name="pu")

                def expt_upd(jb):
                    # transposes + upd matmuls for exp columns of block jb
                    for t in range(4):
                        jc = 4 * jb + t
                        pet = ps.tile([P, P], f32, tag="blk")
                        nc.tensor.transpose(
                            pet[:P, :P], exp_sb[:, jc * P : (jc + 1) * P],
                            ident[:],
                        )
                        nc.scalar.copy(expT[jc][:], pet[:P, :P])
                        if jc >= 1:
                            nc.tensor.matmul(
                                pu[:], expT[jc - 1][:], Vb[jc - 1][:],
                                start=(jc == 1), stop=False,
                            )

                for jb in range(NJ):
                    jsl = slice(jb * 512, (jb + 1) * 512)
                    pss2 = ps_s.tile([P, 512], f32, tag="psSc")
                    psS.append(pss2)
                    n = 0
                    for ic in range(ND):
                        for lh, rh in (
                            (YTh[ic][:], xTh[ic][:, jsl]),
                            (YTl[ic][:], xTh[ic][:, jsl]),
                            (YTh[ic][:], xTl[ic][:, jsl]),
                        ):
                            nc.tensor.matmul(
                                pss2[:], lh, rh,
                                start=(n == 0), stop=(n == 3 * ND - 1),
                            )
                            n += 1
                    s3 = scr.tile([P, 512], f32, tag="scrt")
                    nc.vector.tensor_tensor(
                        out=s3[:], in0=pss2[:], in1=gm[:, jsl], op=OP.mult
                    )
                    nc.vector.reduce_max(cmax[:, jb : jb + 1], s3[:], axis=AX.X)
                    s4 = scr.tile([P, 512], f32, tag="scrt")
                    nc.vector.scalar_tensor_tensor(
                        out=s4[:], in0=pss2[:], scalar=-1.0 / L, in1=gc[:, jsl],
                        op0=OP.mult, op1=OP.mult,
                        accum_out=csum[:, jb : jb + 1],
                    )
                    # softmax exp (no max-subtraction: scores*SCALE is O(10),
                    # exp is fp32-safe)
                    nc.scalar.activation(
                        out=exp_sb[:, jsl], in_=pss2[:], func=ACTF.Exp,
                        bias=0.0, scale=SCALE,
                        accum_out=sume4[:, jb : jb + 1],
                    )
                    if jb >= 1:
                        expt_upd(jb - 1)
                u1 = cnd.tile([P, 1], f32, tag="u1")
                u2 = cnd.tile([P, 1], f32, tag="u2")
                M_cand = cnd.tile([P, 1], f32, tag="M_cand")
                nc.vector.reduce_max(u1[:], cmax[:], axis=AX.X)
                nc.vector.reduce_sum(u2[:], csum[:], axis=AX.X)
                nc.vector.tensor_tensor(out=M_cand[:], in0=u1[:], in1=u2[:], op=OP.add)
                nc.vector.copy_predicated(M_cand[:], invalid[:], negbig[:])
                sume = cnd.tile([P, 1], f32, tag="sume")
                nc.vector.reduce_sum(sume[:], sume4[:], axis=AX.X)
                recip = cnd.tile([P, 1], f32, tag="recip")
                nc.vector.reciprocal(recip[:], sume[:])

                # kick off the candidate top-40 rounds on the vector engine
                # (they gate the scatter), then finish the expT/upd pipeline
                # for the last block concurrently with them
                pmc = ps.tile([1, P], f32, tag="blk")
                nc.tensor.transpose(pmc[:1, :P], M_cand[:], ident[:])
                mcT = cnd.tile([1, P], f32, tag="mcT")
                nc.vector.tensor_copy(mcT[:], pmc[:1, :P])
                etop = cnd.tile([1, NT], f32, tag="etop")
                for r in range(5):
                    nc.vector.max(out=etop[:, 8 * r : 8 * r + 8], in_=mcT[:])
                    if r < 4:
                        nc.vector.match_replace(
                            out=mcT[:], in_to_replace=etop[:, 8 * r : 8 * r + 8],
                            in_values=mcT[:], imm_value=NEG,
                        )

                expt_upd(NJ - 1)
                nc.tensor.matmul(
                    pu[:], expT[NL - 1][:], Vb[NL - 1][:],
                    start=False, stop=True,
                )

                # threshold broadcast + selection (vector rounds already done)
                pte = ps.tile([P, 1], f32, tag="blk")
                nc.tensor.matmul(
                    pte[:P, :1], ones_r1[:], etop[:, NT - 1 : NT], start=True, stop=True
                )
                tebc = cnd.tile([P, 1], f32, tag="tebc")
                nc.vector.tensor_copy(tebc[:], pte[:P, :1])
                sel2 = cnd.tile([P, 1], u8, tag="sel2")
                nc.vector.tensor_tensor(
                    out=sel2[:], in0=M_cand[:], in1=tebc[:], op=OP.is_ge
                )
                scat_f = cnd.tile([P, 1], f32, tag="scat_f")
                nc.vector.tensor_copy(scat_f[:], big9[:])
                nc.vector.copy_predicated(scat_f[:], sel2[:], candq_f[:])
                scat_i = cnd.tile([P, 1], i32, tag="scat_i")
                nc.vector.tensor_copy(scat_i[:], scat_f[:])

                upd = cnd.tile([P, D], f32, tag="upd")
                nc.scalar.activation(
                    out=upd[:], in_=pu[:], func=ACTF.Copy, bias=0.0, scale=recip[:]
                )
                nc.gpsimd.indirect_dma_start(
                    out=ctx_d[:],
                    out_offset=bass.IndirectOffsetOnAxis(ap=scat_i[:, :1], axis=0),
                    in_=upd[:], in_offset=None,
                    bounds_check=L - 1, oob_is_err=False,
                )

    nc.compile()
    return nc


_NC = None


def _get_nc():
    global _NC
    if _NC is None:
        _NC = build()
    return _NC


def _split_bf16(a):
    hi = a.astype(ml_dtypes.bfloat16)
    lo = (a - hi.astype(np.float32)).astype(ml_dtypes.bfloat16)
    return hi, lo


def _row_chunk(w):
    # [512, 512] -> [128, 4*512]: tile dc = rows dc*128..dc*128+127
    return np.concatenate([w[dc * P : (dc + 1) * P, :] for dc in range(4)], axis=1)


def _host_prep(x, Wq, Wk, Wv, index_sample):
    x = np.asarray(x, dtype=np.float32)
    Wq = np.asarray(Wq, dtype=np.float32)
    Wk = np.asarray(Wk, dtype=np.float32)
    Wv = np.asarray(Wv, dtype=np.float32)
    idx = np.asarray(index_sample)

    wqT = np.ascontiguousarray(Wq.T)
    wqh, _ = _split_bf16(wqT)
    wkh, wkl = _split_bf16(np.ascontiguousarray(Wk.T))
    wvh, wvl = _split_bf16(np.ascontiguousarray(Wv.T))
    A = (Wq.T.astype(np.float64) @ Wk.astype(np.float64)).astype(np.float32)

    rows = np.arange(L)[:, None]
    mask01 = np.zeros((L, L), dtype=np.uint8)
    mask01[rows, idx] = 1
    maskb = mask01.astype(ml_dtypes.bfloat16)
    countf = np.zeros((L, L), dtype=np.uint8)
    np.add.at(countf, (rows, idx), 1)

    perm16 = np.zeros((16, 8 * P), dtype=np.float32)
    for f in range(8):
        for p in range(16):
            perm16[p, f * P + p + 16 * f] = 1.0
    identf = np.eye(P, dtype=np.float32)
    qidxf = (np.arange(P, dtype=np.float32)[:, None]
             + 128.0 * np.arange(16, dtype=np.float32)[None, :])
    acat = np.ascontiguousarray(_row_chunk(A))
    shared = {
        "Acat": acat, "mask01": mask01, "maskb": maskb, "countf": countf,
        "perm16": perm16, "identf": identf, "qidxf": qidxf,
    }
    wcat = np.ascontiguousarray(
        np.concatenate(
            [_row_chunk(wqh), _row_chunk(wkh), _row_chunk(wvh)], axis=1
        ).astype(ml_dtypes.bfloat16)
    )
    in_maps = []
    for b in range(B):
        xb = np.ascontiguousarray(x[b])
        xT = np.ascontiguousarray(xb.T)
        xth, xtl = _split_bf16(xT)
        xmean = xb.astype(np.float64).mean(axis=0)
        vmean = (xmean @ Wv.T.astype(np.float64)).astype(np.float32)
        in_maps.append(
            {
                "x_nat": xb,
                "xTh": xth,
                "xTl": xtl,
                "wcat": wcat,
                "vmeanr": vmean.reshape(1, D),
                **shared,
            }
        )
    return in_maps


def kernel(x, Wq, Wk, Wv, index_sample, _trace=False, _result_box=None):
    in_maps = _host_prep(x, Wq, Wk, Wv, index_sample)
    nc = _get_nc()
    res = run_bass_kernel_spmd(nc, in_maps, core_ids=list(range(B)), trace=_trace)
    if _result_box is not None:
        _result_box.append(res)
    out = np.stack([np.asarray(res.results[b]["ctx"]) for b in range(B)], axis=0)
    return out

